# revision 2
# baseline (speedup 1.0000x reference)
"""Trainium2 Bass kernel for nn_DSFE (dual attention + LFE depthwise-conv block).

Sharding: pure data-parallel over batch B=16 across 8 NeuronCores (2 batches
per core), no collectives. Each core runs the whole per-batch network on its
shard.

v2: mixed-precision rewrite of the fp32 baseline.
  - attention branch end-to-end bf16 matmul inputs (PE 1 cyc/row incl. the
    small-free-dim matmuls and all transposes via a bf16 identity; fp32 was
    4 cyc/row small / 2 cyc/row transpose)
  - LFE branch in fp8e4m3 with DoubleRow matmuls (2 contraction rows per
    partition, 0.5 cyc/row): fc1 and fc2 pair adjacent K-chunks; the 3x3
    depthwise conv pairs taps (0,kw)+(2,kw) whose flat shifts differ by 64
    elements (DoubleRow needs 16B-aligned pair strides; row-1 taps stay
    single fp8 matmuls)
  - weights quantized x64 into fp8 (0.02-scale values are subnormal in
    e4m3 otherwise); T stored as 8*T, GELU output as 64*g via the
    x*sigmoid(1.702x) identity so the x64 rides the DVE multiply for free;
    all scales unwound in PSUM-eviction scale factors (512 for conv, 4096
    for fc2)
  - all weights transposed on-device once in the preamble and kept
    resident in SBUF (bf16/fp8 halve the footprint; no DRAM scratch
    round-trip, no per-batch weight reloads)
  - v_sa never materialized: v_proj = Wv_sa @ (F @ X)^T via a rank-16
    side matmul, so QKVV computes only 3C columns
  - biases via DMA-broadcast const tiles + DVE adds at PSUM eviction
    (replaces rank-1 ones matmuls on the PE)
  - conv wrap-column fixups and the z-eviction moved to GPSIMD

Measured numpy-emulated accuracy of this scheme: rel ~1.1e-2 (gate 2e-2).
"""

from contextlib import ExitStack

import numpy as np

import concourse.bass as bass
import concourse.mybir as mybir
import concourse.tile as tile
from concourse.masks import make_identity

FP = mybir.dt.float32
FR = mybir.dt.float32r
BF = mybir.dt.bfloat16
F8 = mybir.dt.float8e4
AF = mybir.ActivationFunctionType
ALU = mybir.AluOpType
DR = mybir.MatmulPerfMode.DoubleRow
ts = bass.ts

B, W_, H_, C = 16, 32, 32, 512
HEADS, PP, HID = 4, 16, 2048
N = H_ * W_            # 1024
D = C // HEADS         # 128
NCORES = 8
BL = B // NCORES       # 2 batches per core
NCH = N // 128         # 8
CCH = C // 128         # 4
OCH = HID // 128       # 16

NORM_EPS = 1e-12
WSC = 64.0             # fp8 weight scale
TSC = 8.0              # fp8 T scale
GSC = 64.0             # fp8 g scale

# conv input layout: rows padded to stride RS=40 (pair deltas stay 16B
# aligned and no shift ever wraps into a neighbouring row), region A = 8*T,
# region B = A shifted by +2 (pairs row-1 taps), region C = constant 8.0
# (pairs the dw-bias "tap" with (1,1)). Tap pair order (blocks 2p, 2p+1 of
# the diag tile): (0,kw)+(2,kw) for kw=0..2, (1,0)+(1,2), (1,1)+bias.
RS = 40
LEAD = 48
AW = 1392                  # region width (A at 0, B at AW, C at 2*AW)
TAP_ORDER = [(0, 0), (2, 0), (0, 1), (2, 1), (0, 2), (2, 2),
             (1, 0), (1, 2), (1, 1)]


def ap3(t2d, off, pair_step, inner):
    """From a 2D tile AP, build a (P, 2, inner) AP with a custom pair
    stride (for DoubleRow over two shifted windows of the same tile)."""
    a = t2d.copy()
    dims = list(a.ap)
    return bass.AP(a.tensor, a.offset + off,
                   [list(dims[0]), [pair_step, 2], [1, inner]])


def ap4(t2d, off, pair_step, nrow):
    """(P, 2, nrow, 32) AP over the padded conv layout: pair stride +
    row-stride RS + contiguous 32 columns."""
    a = t2d.copy()
    dims = list(a.ap)
    return bass.AP(a.tensor, a.offset + off,
                   [list(dims[0]), [pair_step, 2], [RS, nrow], [1, W_]])


def split_multi_waits(nc):
    """This environment's walrus build encodes at most ONE sync wait per
    instruction; re-host excess waits onto same-engine NoOps just before."""
    n_split = 0
    for f in nc.m.functions:
        for blk in f.blocks:
            new = []
            changed = False
            for inst in blk.instructions:
                si = inst.sync_info
                waits = list(si.on_wait) if si and si.on_wait else []
                if len(waits) > 1:
                    n_split += 1
                    changed = True
                    for w in waits[:-1]:
                        new.append(mybir.InstNoOp(
                            name=nc.get_next_instruction_name(),
                            sync_info=mybir.SyncInfo(on_wait=[w], on_update=[]),
                            bass_nofuse=True,
                            engine=inst.engine,
                        ))
                    inst.sync_info = mybir.SyncInfo(
                        on_wait=[waits[-1]],
                        on_update=list(si.on_update) if si.on_update else [],
                    )
                new.append(inst)
            if changed:
                blk.instructions = new
    return n_split


def build(split_waits=True, loop=1, phases="AB"):
    """Build the per-core Bass module (SPMD: identical program on 8 cores)."""
    nc = bass.Bass("TRN2", target_bir_lowering=False, debug=False)

    def din(name, shape):
        return nc.dram_tensor(name, list(shape), FP, kind="ExternalInput").ap()

    # tiny DRAM scratch to round-trip computed bias rows for partition
    # broadcast (DMA broadcast must source from DRAM)
    scratch = dict(
        brf=nc.dram_tensor("brf_s", [1, C], FP).ap(),
    )

    aps = dict(
        x=din("x", (BL, W_, H_, C)),
        qkvv_w=din("qkvv_w", (4 * C, C)),
        e_w=din("E_w", (PP, N)),
        e_b=din("E_b", (PP,)),
        f_w=din("F_w", (PP, N)),
        f_b=din("F_b", (PP,)),
        temp=din("temp", (HEADS, 1, 1)),
        temp2=din("temp2", (HEADS, 1, 1)),
        out_w=din("out_w", (C // 2, C)),
        out_b=din("out_b", (C // 2,)),
        out2_w=din("out2_w", (C // 2, C)),
        out2_b=din("out2_b", (C // 2,)),
        fc1_w=din("fc1_w", (HID, C)),
        fc1_b=din("fc1_b", (HID,)),
        dw_w=din("dw_w", (HID, 1, 3, 3)),
        dw_b=din("dw_b", (HID,)),
        fc2_w=din("fc2_w", (C, HID)),
        fc2_b=din("fc2_b", (C,)),
        y=nc.dram_tensor("y", [BL, W_, H_, C], FP, kind="ExternalOutput").ap(),
    )

    with tile.TileContext(nc) as tc:
        _emit(nc, tc, aps, scratch, loop, phases)

    if split_waits:
        split_multi_waits(nc)
    return nc


def _transpose_weight(nc, tc, ident, nat_ap, dst_fn, label, scale=None):
    """PE-transpose a natural (R, K) weight; for each K-chunk ci the
    transposed (128, R) block is evicted through dst_fn(ci) -> AP.
    scale!=None evicts via ACT with that scale (for fp8 targets)."""
    dve = nc.vector
    pe = nc.tensor
    R, K = nat_ap.shape
    RC, KC = R // 128, K // 128
    with ExitStack() as es:
        nat_pool = es.enter_context(tc.tile_pool(name=f"tw_nat_{label}", bufs=2))
        ps_pool = es.enter_context(
            tc.tile_pool(name=f"tw_ps_{label}", bufs=4, space="PSUM"))
        for rj in range(RC):
            nat = nat_pool.tile([128, K], FP, tag="nat", name=f"{label}nat{rj}")
            nc.sync.dma_start(nat[:], nat_ap[rj * 128:(rj + 1) * 128])
            for ci in range(KC):
                ps = ps_pool.tile([128, 128], FP, tag="tp", name=f"{label}tp")
                pe.transpose(ps[:], nat[:, ts(ci, 128)], ident[:])
                dst = dst_fn(ci)[:, rj * 128:(rj + 1) * 128]
                if scale is None:
                    dve.tensor_copy(dst, ps[:])
                else:
                    nc.scalar.activation(dst, ps[:], AF.Identity, scale=scale)


def _emit(nc, tc, aps, scratch, loop=1, phases="AB"):
    dve = nc.vector
    pe = nc.tensor

    with ExitStack() as es:
        const = es.enter_context(tc.tile_pool(name="const", bufs=1))

        ident = const.tile([128, 128], FP)
        make_identity(nc, ident)
        identb = const.tile([128, 128], BF)
        make_identity(nc, identb)
        ones_bf = const.tile([1, 128], BF)
        dve.memset(ones_bf[:], 1.0)

        # ---- resident transposed weights
        # qw[i]: (c-chunk i, 4C) bf16  (v_sa quarter used only by v_proj)
        qw = [const.tile([128, 4 * C], BF, tag=f"qw{i}", name=f"qw{i}")
              for i in range(CCH)]
        _transpose_weight(nc, tc, ident, aps["qkvv_w"],
                          lambda ci: qw[ci][:], "qkvv")
        # ft8[j]: fc1T fp8 x64 pair tiles (128, 2*HID): slot s = c-chunk 2j+s
        ft8 = [const.tile([128, 2 * HID], F8, tag=f"ft8{j}", name=f"ft8{j}")
               for j in range(CCH // 2)]
        _transpose_weight(
            nc, tc, ident, aps["fc1_w"],
            lambda ci: ft8[ci // 2][:, (ci % 2) * HID:(ci % 2 + 1) * HID],
            "fc1", scale=WSC)
        # f2t8[j]: fc2T fp8 x64 pair tiles (128, 2*C): slot s = o-chunk 2j+s
        f2t8 = [const.tile([128, 2 * C], F8, tag=f"f2t8{j}", name=f"f2t8{j}")
                for j in range(OCH // 2)]
        _transpose_weight(
            nc, tc, ident, aps["fc2_w"],
            lambda ci: f2t8[ci // 2][:, (ci % 2) * C:(ci % 2 + 1) * C],
            "fc2", scale=WSC)
        # out_w^T / out2_w^T chunks (c-part, j) bf16
        owt = [const.tile([128, C // 2], BF, tag=f"owt{q}", name=f"owt{q}")
               for q in range(CCH)]
        owt2 = [const.tile([128, C // 2], BF, tag=f"owt2{q}", name=f"owt2{q}")
                for q in range(CCH)]
        _transpose_weight(nc, tc, ident, aps["out_w"],
                          lambda ci: owt[ci][:], "outw")
        _transpose_weight(nc, tc, ident, aps["out2_w"],
                          lambda ci: owt2[ci][:], "outw2")

        # E^T as (n-part r, n-chunk k, p) bf16; F^T in m-order (m = 32w+h,
        # matching the natural x row order) since it only contracts raw X
        ewt = const.tile([128, NCH, PP], BF)
        fwt = const.tile([128, NCH, PP], BF)
        with ExitStack() as ef:
            ef_pool = ef.enter_context(tc.tile_pool(name="ef_nat", bufs=2))
            ef_ps = ef.enter_context(tc.tile_pool(name="ef_ps", bufs=4, space="PSUM"))
            e_nat = ef_pool.tile([PP, N], FP, tag="nat", name="e_nat")
            nc.sync.dma_start(e_nat[:], aps["e_w"][:])
            f_nat = ef_pool.tile([PP, N], FP, tag="nat", name="f_nat")
            fv3 = f_nat[:].rearrange("p (w h) -> p w h", h=H_)
            sv3 = aps["f_w"].rearrange("p (h w) -> p h w", w=W_)
            for w in range(W_):
                nc.sync.dma_start(fv3[:, w, :], sv3[:, :, w])
            for k in range(NCH):
                ps = ef_ps.tile([128, PP], FP, tag="tp", name="e_tp")
                pe.transpose(ps[:], e_nat[:, ts(k, 128)], ident[0:PP, 0:PP])
                dve.tensor_copy(ewt[:, k, :], ps[:])
                ps2 = ef_ps.tile([128, PP], FP, tag="tp", name="f_tp")
                pe.transpose(ps2[:], f_nat[:, ts(k, 128)], ident[0:PP, 0:PP])
                dve.tensor_copy(fwt[:, k, :], ps2[:])

        # temp/temp2 broadcast to all 128 partitions: (128, HEADS)
        tcol = const.tile([128, HEADS], FP)
        nc.sync.dma_start(
            tcol[:], aps["temp"].rearrange("h a b -> (a b) h").to_broadcast((128, HEADS)))
        t2col = const.tile([128, HEADS], FP)
        nc.sync.dma_start(
            t2col[:], aps["temp2"].rearrange("h a b -> (a b) h").to_broadcast((128, HEADS)))

        # E/F bias rows broadcast over partitions
        ebc = const.tile([128, PP], FP)
        nc.sync.dma_start(ebc[:], aps["e_b"].unsqueeze(0).to_broadcast((128, PP)))
        fbc = const.tile([128, PP], FP)
        nc.sync.dma_start(fbc[:], aps["f_b"].unsqueeze(0).to_broadcast((128, PP)))

        # OUTA free-dim bias row concat(out_b,out2_b)+fc2_b, broadcast via
        # a DRAM round-trip (SBUF-sourced partition broadcast is illegal)
        br_cat = const.tile([1, C], FP)
        nc.sync.dma_start(br_cat[:, 0:C // 2], aps["out_b"].unsqueeze(0))
        nc.sync.dma_start(br_cat[:, C // 2:C], aps["out2_b"].unsqueeze(0))
        fcb_row = const.tile([1, C], FP)
        nc.sync.dma_start(fcb_row[:], aps["fc2_b"].unsqueeze(0))
        brf_row = const.tile([1, C], FP)
        dve.tensor_add(brf_row[:], br_cat[:], fcb_row[:])
        nc.sync.dma_start(scratch["brf"], brf_row[:])
        brf128 = const.tile([128, C], FP)
        nc.sync.dma_start(brf128[:], scratch["brf"].to_broadcast((128, C)))

        # per-partition (o) bias/weight columns for the LFE branch
        fb1 = const.tile([128, OCH], FP)
        nc.sync.dma_start(fb1[:], aps["fc1_b"].rearrange("(k r) -> r k", r=128))
        fb1_8 = const.tile([128, OCH], FP)
        dve.tensor_scalar_mul(fb1_8[:], fb1[:], TSC)
        dwb = const.tile([128, OCH], FP)
        nc.sync.dma_start(dwb[:], aps["dw_b"].rearrange("(k r) -> r k", r=128))
        dww = const.tile([128, OCH * 9], FP)
        for oc in range(OCH):
            nc.sync.dma_start(
                dww[:, oc * 9:(oc + 1) * 9],
                aps["dw_w"][oc * 128:(oc + 1) * 128].rearrange("r a kh kw -> r (a kh kw)"))
        # x64 weights reordered to TAP_ORDER + the x64 dw-bias as block 9
        dwwr = const.tile([128, OCH * 10], FP)
        dv3 = dww[:].rearrange("p (k t) -> p k t", t=9)
        dr3 = dwwr[:].rearrange("p (k t) -> p k t", t=10)
        for t_new, (kh, kw) in enumerate(TAP_ORDER):
            dve.scalar_tensor_tensor(
                dr3[:, :, t_new], dv3[:, :, 3 * kh + kw], WSC,
                dv3[:, :, 3 * kh + kw], op0=ALU.mult, op1=ALU.bypass)
        dve.scalar_tensor_tensor(
            dr3[:, :, 9], dwb[:], WSC, dwb[:], op0=ALU.mult, op1=ALU.bypass)
        # resident fp8 diag(w) tiles, 10 blocks each (9 taps + bias)
        dgs = [const.tile([128, 10 * 128], F8, tag=f"dg{oc}", name=f"dg{oc}")
               for oc in range(OCH)]
        for oc in range(OCH):
            dve.tensor_mul(
                dgs[oc][:].rearrange("p (t j) -> p t j", j=128),
                identb[:].unsqueeze(1).to_broadcast((128, 10, 128)),
                dwwr[:, oc * 10:(oc + 1) * 10].unsqueeze(2)
                    .to_broadcast((128, 10, 128)))

        cst = dict(ident=ident, identb=identb, ones_bf=ones_bf,
                   tcol=tcol, t2col=t2col,
                   ewt=ewt, fwt=fwt, ebc=ebc, fbc=fbc, brf128=brf128,
                   fb1_8=fb1_8, dgs=dgs,
                   qw=qw, ft8=ft8, f2t8=f2t8, owt=owt, owt2=owt2)

        # cross-batch x staging: DMA + bf16 conversion for batch i+1 are
        # emitted in the middle of batch i (prefetch), double-buffered
        xnb_pool = es.enter_context(tc.tile_pool(name="xnbp", bufs=1))
        xn_pool = es.enter_context(tc.tile_pool(name="xnstage", bufs=3))
        xnb_sets = [[xnb_pool.tile([128, C], BF, tag=f"xnb{s}_{kk}",
                                   name=f"xnb{s}_{kk}") for kk in range(NCH)]
                    for s in range(2)]
        seq = [(rep, b) for rep in range(loop) for b in range(BL)]

        def loadX(idx):
            if idx >= len(seq):
                return
            _, b = seq[idx]
            xflat = aps["x"][b].rearrange("w h c -> (w h) c")
            for kk in range(NCH):
                xn = nc_tile = xn_pool.tile([128, C], FP, tag="xn", name="xn")
                nc.sync.dma_start(xn[:], xflat[kk * 128:(kk + 1) * 128])
                nc.scalar.activation(xnb_sets[idx % 2][kk][:], xn[:], AF.Identity)

        loadX(0)
        for idx, (rep, b) in enumerate(seq):
            _emit_batch(nc, tc, b, aps, cst,
                        label=f"{rep}_{b}", phases=phases,
                        xnb=xnb_sets[idx % 2],
                        prefetch=(lambda i=idx: loadX(i + 1)))


def _emit_batch(nc, tc, b, aps, cst, label=None, phases="AB", xnb=None,
                prefetch=None):
    if label is None:
        label = str(b)
    act = nc.scalar
    dve = nc.vector
    gps = nc.gpsimd
    pe = nc.tensor
    identb = cst["identb"]
    qw = cst["qw"]

    with ExitStack() as bs:
        xc_pool = bs.enter_context(tc.tile_pool(name=f"xc{label}", bufs=1))
        outa_pool = bs.enter_context(tc.tile_pool(name=f"outa{label}", bufs=1))

        # ---- XC (c, n), n = 32h + w: load x[b] rows naturally (m = 32w + h
        # order), bf16-ize, PE-transpose 128x128 blocks, and fix the h/w swap
        # with a permuted SBUF write AP on the evicts (bf16 + fp8 copies).
        # The same pass accumulates XF = F @ X (16, C) for the v_proj path.
        xc = [xc_pool.tile([128, N], BF, tag=f"xc{i}", name=f"xc{i}")
              for i in range(CCH)]
        x8p = [xc_pool.tile([128, 2 * N], F8, tag=f"x8p{j}", name=f"x8p{j}")
               for j in range(CCH // 2)]
        xft = xc_pool.tile([128, CCH * PP], BF, tag="xft", name="xft")
        with ExitStack() as xl:
            xn_ps = xl.enter_context(
                tc.tile_pool(name=f"xnps{label}", bufs=4, space="PSUM"))
            xf_psp = xl.enter_context(
                tc.tile_pool(name=f"xfps{label}", bufs=1, space="PSUM"))
            xf_ps = xf_psp.tile([PP, C], FP, tag="xf", name="xf_ps")
            for kk in range(NCH):
                pe.matmul(xf_ps[:], cst["fwt"][:, kk, :], xnb[kk][:],
                          start=(kk == 0), stop=(kk == NCH - 1),
                          skip_group_check=True)
                for i in range(CCH):
                    ps = xn_ps.tile([128, 128], BF, tag="xt", name="xt")
                    pe.transpose(ps[:], xnb[kk][:, ts(i, 128)], identb[:])
                    xcv = xc[i][:].rearrange("p (h w) -> p h w", w=W_)
                    dve.tensor_copy(
                        xcv[:, :, 4 * kk:4 * kk + 4].transpose([0, 2, 1]),
                        ps[:].rearrange("p (a b) -> p a b", b=32))
            xf_sb = xc_pool.tile([PP, C], BF, tag="xf_sb", name="xf_sb")
            dve.tensor_copy(xf_sb[:], xf_ps[:])
            for i in range(CCH):
                psx = xn_ps.tile([128, PP], BF, tag="xft", name="xft_ps", bufs=2)
                pe.transpose(psx[:], xf_sb[:, ts(i, 128)], identb[0:PP, 0:PP])
                dve.tensor_copy(xft[:, ts(i, PP)], psx[:])
            # fp8 copy of XC for the fc1 DoubleRow path, on idle GPSIMD
            for i in range(CCH):
                gps.tensor_copy(x8p[i // 2][:, (i % 2) * N:(i % 2 + 1) * N],
                                xc[i][:])

        outa = [outa_pool.tile([128, C], FP, tag=f"outa{m}", name=f"outa{m}")
                for m in range(NCH)]
        if phases != "AB":
            for m in range(NCH):
                nc.vector.memset(outa[m][:], 0.0)
        if "B" not in phases:
            ydst0 = aps["y"][b].rearrange("w h c -> h w c")
            nc.sync.dma_start(ydst0[0], outa[0][0:32, :])

        # ======================= phase A: attention =======================
        if "A" in phases:
          with ExitStack() as pa:
            qkvv_pool = pa.enter_context(tc.tile_pool(name=f"qkvv{label}", bufs=1))
            # PSUM budget (8 banks): big bf16 (128,N) x2 = 2, f512 fp32
            # (128,512) x4 = 4, at bf16 (16,N) x1 = 1, med small x4 ~ 1
            ps_big = pa.enter_context(tc.tile_pool(name=f"psb{label}", bufs=3, space="PSUM"))
            ps_f512 = pa.enter_context(tc.tile_pool(name=f"psf{label}", bufs=2, space="PSUM"))
            ps_at = pa.enter_context(tc.tile_pool(name=f"psat{label}", bufs=1, space="PSUM"))
            ps_med = pa.enter_context(tc.tile_pool(name=f"psm{label}", bufs=2, space="PSUM"))

            # ---- QKVV (n, 2C) bf16: only q|k columns; v_ca^T is computed
            # directly in transposed layout per head, v_sa via the XF path
            qkvv = [qkvv_pool.tile([128, 2 * C], BF, tag=f"qkvv{k}", name=f"qkvv{k}")
                    for k in range(NCH)]
            for k in range(NCH):
                for j in range(2):
                    ps = ps_f512.tile([128, 512], FP, tag="f512")
                    for i in range(CCH):
                        pe.matmul(ps[:], xc[i][:, ts(k, 128)],
                                  qw[i][:, ts(j, 512)],
                                  start=(i == 0), stop=(i == CCH - 1))
                    dve.tensor_copy(qkvv[k][:, ts(j, 512)], ps[:])

            # prefetch the next batch's x (DMA + bf16 conversion) so its
            # XC stage starts without the load latency
            if prefetch is not None:
                prefetch()
                prefetch = None

            with ExitStack() as ph:
                at_pool = pa.enter_context(tc.tile_pool(name=f"at{label}", bufs=2))
                xca_pool = pa.enter_context(tc.tile_pool(name=f"xca{label}", bufs=1))
                xsa_pool = pa.enter_context(tc.tile_pool(name=f"xsa{label}", bufs=1))

                xsa = [xsa_pool.tile([128, N], BF, tag=f"xsa{q}", name=f"xsa{q}")
                       for q in range(CCH)]
                xca = []

                def headA(h):
                    """Transposes + norm chains for head h (PE: 16
                    transposes; ACT/DVE: the sqrt/recip chains)."""
                    qc, kc = h * 128, C + h * 128
                    s = {}
                    qT_ps = ps_big.tile([128, N], BF, tag="big")
                    for k in range(NCH):
                        pe.transpose(qT_ps[:, ts(k, 128)],
                                     qkvv[k][:, qc:qc + 128], identb[:])
                    kT_ps = ps_big.tile([128, N], BF, tag="big")
                    for k in range(NCH):
                        pe.transpose(kT_ps[:, ts(k, 128)],
                                     qkvv[k][:, kc:kc + 128], identb[:])
                    sq = at_pool.tile([128, N], BF, tag="sq")
                    ssq = at_pool.tile([128, 1], FP, tag="ssq")
                    act.activation(sq[:], qT_ps[:], AF.Square, accum_out=ssq[:])
                    nrmq = at_pool.tile([128, 1], FP, tag="nrmq")
                    act.activation(nrmq[:], ssq[:], AF.Sqrt)
                    invq = at_pool.tile([128, 1], FP, tag="invq")
                    dve.reciprocal(invq[:], nrmq[:])
                    invq_t = at_pool.tile([128, 1], FP, tag="invq_t")
                    dve.tensor_mul(invq_t[:], invq[:], cst["tcol"][:, h:h + 1])
                    qn_t = at_pool.tile([128, N], BF, tag="qn_t")
                    act.activation(qn_t[:], qT_ps[:], AF.Identity, scale=invq[:])
                    sqk = at_pool.tile([128, N], BF, tag="sq")
                    ssqk = at_pool.tile([128, 1], FP, tag="ssqk")
                    act.activation(sqk[:], kT_ps[:], AF.Square, accum_out=ssqk[:])
                    nrmk = at_pool.tile([128, 1], FP, tag="nrmk")
                    act.activation(nrmk[:], ssqk[:], AF.Sqrt)
                    invk = at_pool.tile([128, 1], FP, tag="invk")
                    dve.reciprocal(invk[:], nrmk[:])
                    invk_b = at_pool.tile([128, 1], BF, tag="invk_b")
                    dve.tensor_copy(invk_b[:], invk[:])
                    s.update(invq_t=invq_t, qn_t=qn_t, invk_b=invk_b)
                    return s

                def colsc_pe(h, s):
                    """Column-scale tile via rank-1 matmul; emitted late so
                    the PE never stalls on the invk chain."""
                    ikr_ps = ps_med.tile([1, 128], BF, tag="med")
                    pe.transpose(ikr_ps[:], s["invk_b"][:], identb[:])
                    ikr = at_pool.tile([1, 128], BF, tag="ikr")
                    dve.tensor_copy(ikr[:], ikr_ps[:])
                    colsc_ps = ps_med.tile([128, 128], FP, tag="med")
                    pe.matmul(colsc_ps[:], cst["ones_bf"][:], ikr[:],
                              start=True, stop=True)
                    colsc = at_pool.tile([128, 128], FP, tag="colsc")
                    dve.tensor_copy(colsc[:], colsc_ps[:])
                    s["colsc"] = colsc

                def headB(h, s):
                    qc = h * 128            # q columns in QKVV
                    kc = C + h * 128        # k columns
                    vc = 2 * C + h * 128    # v_ca columns in qkvv_w^T

                    # ---- CA scores S0 = q @ k^T
                    s_ps = ps_med.tile([128, 128], FP, tag="med")
                    for k in range(NCH):
                        pe.matmul(s_ps[:], qkvv[k][:, qc:qc + 128],
                                  qkvv[k][:, kc:kc + 128],
                                  start=(k == 0), stop=(k == NCH - 1))
                    s_sb = at_pool.tile([128, 128], FP, tag="s_sb")
                    dve.scalar_tensor_tensor(s_sb[:], s_ps[:], s["invq_t"][:],
                                             s["colsc"][:], op0=ALU.mult,
                                             op1=ALU.mult)

                    # ---- SA: k_proj (d, p) from qkvv
                    kp_ps = ps_med.tile([128, PP], FP, tag="med")
                    for k in range(NCH):
                        pe.matmul(kp_ps[:], qkvv[k][:, kc:kc + 128],
                                  cst["ewt"][:, k, :],
                                  start=(k == 0), stop=(k == NCH - 1))
                    kp_sb = at_pool.tile([128, PP], BF, tag="kp_sb")
                    dve.tensor_add(kp_sb[:], kp_ps[:], cst["ebc"][:])

                    # ---- v_ca^T (e, n) computed directly
                    vt_sb = at_pool.tile([128, N], BF, tag="vt_sb")
                    for u in range(2):
                        vt_ps = ps_f512.tile([128, 512], FP, tag="f512")
                        for i in range(CCH):
                            pe.matmul(vt_ps[:], qw[i][:, vc:vc + 128],
                                      xc[i][:, ts(u, 512)],
                                      start=(i == 0), stop=(i == CCH - 1))
                        dve.tensor_copy(vt_sb[:, ts(u, 512)], vt_ps[:])

                    # ---- CA row softmax (1/sum folded into x_ca evict)
                    negmax = at_pool.tile([128, 1], FP, tag="negmax")
                    dve.tensor_reduce(negmax[:], s_sb[:], axis=mybir.AxisListType.X,
                                      op=ALU.max, negate=True)
                    e_sb = at_pool.tile([128, 128], BF, tag="e_sb")
                    sume = at_pool.tile([128, 1], FP, tag="sume")
                    act.activation(e_sb[:], s_sb[:], AF.Exp, bias=negmax[:],
                                   accum_out=sume[:])
                    rex = at_pool.tile([128, 1], FP, tag="rex")
                    dve.reciprocal(rex[:], sume[:])

                    # ---- v_proj from Wv_sa @ XF^T
                    vp_ps = ps_med.tile([128, PP], FP, tag="med")
                    for i in range(CCH):
                        pe.matmul(vp_ps[:],
                                  qw[i][:, 3 * C + h * 128:3 * C + (h + 1) * 128],
                                  xft[:, ts(i, PP)],
                                  start=(i == 0), stop=(i == CCH - 1))
                    vp_sb = at_pool.tile([128, PP], BF, tag="vp_sb")
                    dve.tensor_add(vp_sb[:], vp_ps[:], cst["fbc"][:])
                    vpt_ps = ps_med.tile([16, 128], BF, tag="med")
                    pe.transpose(vpt_ps[:], vp_sb[:], identb[:])
                    vpt_sb = at_pool.tile([16, 128], BF, tag="vpt_sb")
                    dve.tensor_copy(vpt_sb[:], vpt_ps[:])

                    et_ps = ps_med.tile([128, 128], BF, tag="med")
                    pe.transpose(et_ps[:], e_sb[:], identb[:])
                    et_sb = at_pool.tile([128, 128], BF, tag="et_sb")
                    dve.tensor_copy(et_sb[:], et_ps[:])

                    # ---- x_ca (d, n) = (1/sum) * exp(S)^T.T @ v_ca^T
                    xca_h = xca_pool.tile([128, N], BF, tag=f"xca{h}")
                    for u in range(2):
                        xca_ps = ps_f512.tile([128, 512], FP, tag="f512")
                        pe.matmul(xca_ps[:], et_sb[:],
                                  vt_sb[:, ts(u, 512)], start=True, stop=True)
                        act.activation(xca_h[:, ts(u, 512)], xca_ps[:],
                                       AF.Identity, scale=rex[:])
                    xca.append(xca_h)

                    # ---- A0 (n, p) per n-chunk, all 8 in one (128, 8, 16)
                    a_ps = ps_med.tile([128, 128], FP, tag="med")
                    a3 = a_ps[:].rearrange("p (k s) -> p k s", s=PP)
                    for k in range(NCH):
                        pe.matmul(a3[:, k, :], s["qn_t"][:, ts(k, 128)],
                                  kp_sb[:], start=True, stop=True)

                    # ---- segmented softmax over p (free-dim broadcasts)
                    amax = at_pool.tile([128, NCH], FP, tag="amax")
                    dve.tensor_reduce(amax[:], a3, axis=mybir.AxisListType.X,
                                      op=ALU.max)
                    zt = at_pool.tile([128, 128], BF, tag="zt")
                    zt3 = zt[:].rearrange("p (k s) -> p k s", s=PP)
                    dve.tensor_sub(zt3, a3,
                                   amax[:].unsqueeze(2).to_broadcast((128, NCH, PP)))
                    ez = at_pool.tile([128, 128], BF, tag="ez")
                    act.activation(ez[:], zt[:], AF.Exp, scale=cst["t2col"][:, h:h + 1])
                    ez3 = ez[:].rearrange("p (k s) -> p k s", s=PP)
                    esum = at_pool.tile([128, NCH], FP, tag="esum")
                    dve.tensor_reduce(esum[:], ez3, axis=mybir.AxisListType.X,
                                      op=ALU.add)
                    rsum = at_pool.tile([128, NCH], FP, tag="rsum")
                    dve.reciprocal(rsum[:], esum[:])
                    attn = at_pool.tile([128, 128], BF, tag="attn")
                    attn3 = attn[:].rearrange("p (k s) -> p k s", s=PP)
                    dve.tensor_mul(attn3, ez3,
                                   rsum[:].unsqueeze(2).to_broadcast((128, NCH, PP)))

                    # ---- attn^T (p, n)
                    at_ps = ps_at.tile([16, N], BF, tag="at")
                    for k in range(NCH):
                        pe.transpose(at_ps[:, ts(k, 128)], attn3[:, k, :], identb[:])
                    at_sb = at_pool.tile([16, N], BF, tag="at_sb")
                    dve.tensor_copy(at_sb[:], at_ps[:])

                    # ---- x_sa in scrambled (c'=n%512, n'=8d+2h+s) layout,
                    # 4 chunks per f512 PSUM tile
                    for sdx in range(2):
                        xs_ps = ps_f512.tile([128, 512], FP, tag="f512")
                        for q in range(CCH):
                            k = 4 * sdx + q
                            pe.matmul(xs_ps[:, ts(q, 128)], at_sb[:, ts(k, 128)],
                                      vpt_sb[:], start=True, stop=True,
                                      skip_group_check=True)
                        for q in range(CCH):
                            dst = xsa[q][:].rearrange("p (d e) -> p d e", e=8)[:, :, 2 * h + sdx]
                            dve.tensor_copy(dst, xs_ps[:, ts(q, 128)])

                # two-head software pipeline: head h+1's transposes/norm
                # chains overlap head h's main block on the other engines
                sts = {0: headA(0)}
                for h in range(HEADS):
                    if h + 1 < HEADS:
                        sts[h + 1] = headA(h + 1)
                    colsc_pe(h, sts[h])
                    headB(h, sts.pop(h))

                # ---- OUTA (n, 512) = [x_sa@out_w^T | x_ca@out2_w^T] (+bias
                # at the DVE evict)
                for m in range(NCH):
                    o_ps = ps_f512.tile([128, C], FP, tag="f512")
                    for q in range(CCH):
                        pe.matmul(o_ps[:, 0:C // 2], xsa[q][:, ts(m, 128)],
                                  cst["owt"][q][:], start=(q == 0), stop=(q == CCH - 1),
                                  skip_group_check=True)
                    for h in range(HEADS):
                        pe.matmul(o_ps[:, C // 2:C], xca[h][:, ts(m, 128)],
                                  cst["owt2"][h][:], start=(h == 0), stop=(h == HEADS - 1),
                                  skip_group_check=True)
                    dve.tensor_add(outa[m][:], o_ps[:, 0:C], cst["brf128"][:])

        # ======================= phase B: LFE =======================
        if prefetch is not None:
            prefetch()
            prefetch = None
        if "B" in phases:
          with ExitStack() as pb:
            g_pool = pb.enter_context(tc.tile_pool(name=f"g{label}", bufs=1))
            # PSUM: t1/f_ps tag "big" 2 banks x2 bufs + conv (128,1280) 3
            # banks x1 = 7 banks
            ps_big2 = pb.enter_context(tc.tile_pool(name=f"psb2{label}", bufs=2, space="PSUM"))
            ps_conv = pb.enter_context(tc.tile_pool(name=f"psc{label}", bufs=1, space="PSUM"))
            # g8 pair tiles (128, 2, N) fp8: slot = oc % 2
            g8p = [g_pool.tile([128, 2 * N], F8, tag=f"g8p{j}", name=f"g8p{j}")
                   for j in range(OCH // 2)]

            with ExitStack() as pf1:
                conv_pool = pf1.enter_context(tc.tile_pool(name=f"conv{label}", bufs=2))
                # two explicit padded-T buffers alternated across ocs (ONE
                # logical tensor each, so the once-per-batch pad/C memsets
                # stay visible to every oc's reads)
                t_sb2 = [conv_pool.tile([128, 3 * AW], F8, tag=f"tsb{i}",
                                        name=f"tsb{i}") for i in range(2)]
                for i in range(2):
                    gps.memset(t_sb2[i][:, 0:2 * AW], 0.0)
                    gps.memset(t_sb2[i][:, 2 * AW:3 * AW], TSC)
                t_sbs = {}

                def emit_fc1(oc):
                    # fc1 via fp8 DoubleRow (c-chunk pairs); T stored as 8*T
                    # into padded region A (row stride RS)
                    t1_ps = ps_big2.tile([128, N], FP, tag="big")
                    for u in (0, 512):
                        for j in range(CCH // 2):
                            pe.matmul(
                                t1_ps[:, u:u + 512],
                                cst["ft8"][j][:]
                                    .rearrange("p (i f) -> p i f", i=2)[:, :, ts(oc, 128)],
                                ap3(x8p[j][:], u, N, 512),
                                start=(j == 0), stop=(j == CCH // 2 - 1),
                                perf_mode=DR)
                    t_sb = t_sb2[oc % 2]
                    # regions A and B (= A shifted +2, pairs the row-1 taps)
                    # both written straight from the fc1 PSUM, one on ACT
                    # and one on DVE (alternating by oc parity for balance)
                    dsts = [LEAD, AW + LEAD - 2]
                    if oc % 2:
                        dsts.reverse()
                    act.activation(
                        t_sb[:, dsts[0]:dsts[0] + RS * H_]
                            .rearrange("p (h q) -> p h q", q=RS)[:, :, 0:W_],
                        t1_ps[:].rearrange("p (h w) -> p h w", w=W_),
                        AF.Identity, scale=TSC / WSC,
                        bias=cst["fb1_8"][:, oc:oc + 1])
                    dve.scalar_tensor_tensor(
                        t_sb[:, dsts[1]:dsts[1] + RS * H_]
                            .rearrange("p (h q) -> p h q", q=RS)[:, :, 0:W_],
                        t1_ps[:].rearrange("p (h w) -> p h w", w=W_),
                        TSC / WSC,
                        cst["fb1_8"][:, oc:oc + 1].unsqueeze(2)
                            .to_broadcast((128, H_, W_)),
                        op0=ALU.mult, op1=ALU.add)
                    t_sbs[oc] = t_sb

                def emit_conv(oc):
                    # 3x3 depthwise conv + dw-bias in DoubleRow matmuls over
                    # the padded layout; the output stays in PADDED (RS-wide
                    # row) coordinates in PSUM so every pair read is one
                    # contiguous 320-col window (8 image rows per matmul).
                    # Pairs: (0,kw)+(2,kw) at delta 2*RS; (1,0)+(1,2) pairs
                    # A with B (delta AW); (1,1)+bias pairs A with the
                    # constant-8.0 region C. PSUM holds 512*(conv + dw_b).
                    t_sb = t_sbs.pop(oc)
                    dg = cst["dgs"][oc]
                    conv_ps = ps_conv.tile([128, RS * H_], FP, tag="conv",
                                           name="conv_ps")
                    deltas = [(LEAD - RS - 1, 2 * RS), (LEAD - RS, 2 * RS),
                              (LEAD - RS + 1, 2 * RS), (LEAD - 1, AW),
                              (LEAD, 2 * AW)]
                    # chunks bank-aligned (matmul out must stay in one bank);
                    # reads stay contiguous for any padded-col range
                    for c0, cw in ((0, 512), (512, 512), (1024, 256)):
                        for p, (off0, dlt) in enumerate(deltas):
                            pe.matmul(
                                conv_ps[:, c0:c0 + cw],
                                dg[:].rearrange("p (i j) -> p i j", i=10)
                                    [:, 2 * p:2 * p + 2, :],
                                ap3(t_sb[:], off0 + c0, dlt, cw),
                                start=(p == 0), stop=(p == len(deltas) - 1),
                                perf_mode=DR, skip_group_check=True)

                    # GELU via x*sigmoid(1.702x); g stored as 64*g fp8, the
                    # z factor read straight from PSUM in the DVE multiply
                    cpv = conv_ps[:].rearrange("p (h q) -> p h q", q=RS)[:, :, 0:W_]
                    sg = conv_pool.tile([128, N], BF, tag="sg", bufs=2)
                    act.activation(sg[:].rearrange("p (h w) -> p h w", w=W_),
                                   cpv, AF.Sigmoid, scale=1.702 / (TSC * WSC))
                    dve.scalar_tensor_tensor(
                        g8p[oc // 2][:, (oc % 2) * N:(oc % 2 + 1) * N]
                            .rearrange("p (h w) -> p h w", w=W_),
                        cpv, GSC / (TSC * WSC),
                        sg[:].rearrange("p (h w) -> p h w", w=W_),
                        op0=ALU.mult, op1=ALU.mult)

                # two-stage software pipeline: fc1(oc+1) issues on the PE
                # while ACT evicts 8*T for conv(oc), hiding the handoff
                for oc in range(OCH):
                    emit_fc1(oc)
                    if oc > 0:
                        emit_conv(oc - 1)
                emit_conv(OCH - 1)

            # ---- fc2 (fp8 DoubleRow over o-chunk pairs) + OUTA -> y
            with ExitStack() as pf2:
                fin_pool = pf2.enter_context(tc.tile_pool(name=f"fin{label}", bufs=2))

                ydst = aps["y"][b].rearrange("w h c -> h w c")  # (H, W, C)
                for m in range(NCH):
                    f_ps2 = ps_big2.tile([128, N], FP, tag="big")
                    f_ps = f_ps2[:, 0:C]
                    for j in range(OCH // 2):
                        pe.matmul(
                            f_ps,
                            g8p[j][:].rearrange("p (i f) -> p i f", i=2)[:, :, ts(m, 128)],
                            cst["f2t8"][j][:].rearrange("p (i f) -> p i f", i=2),
                            start=(j == 0), stop=(j == OCH // 2 - 1),
                            perf_mode=DR)
                    fin = fin_pool.tile([128, C], FP, tag="fin")
                    dve.scalar_tensor_tensor(fin[:], f_ps, 1.0 / (GSC * WSC),
                                             outa[m][:], op0=ALU.mult, op1=ALU.add)
                    for g in range(4):
                        h_row = 4 * m + g
                        nc.sync.dma_start(ydst[h_row], fin[32 * g:32 * (g + 1), :])


_BUILD_CACHE = {}


def _get_nc():
    if "nc" not in _BUILD_CACHE:
        _BUILD_CACHE["nc"] = build()
    return _BUILD_CACHE["nc"]


def kernel(**inputs):
    from concourse.bass_utils import run_bass_kernel_spmd

    def f32(a):
        return np.ascontiguousarray(np.asarray(a, dtype=np.float32))

    x = f32(inputs["x"])
    assert x.shape == (B, W_, H_, C), x.shape
    common = {k: f32(inputs[k]) for k in
              ("qkvv_w", "E_w", "E_b", "F_w", "F_b", "temp", "temp2",
               "out_w", "out_b", "out2_w", "out2_b",
               "fc1_w", "fc1_b", "dw_w", "dw_b", "fc2_w", "fc2_b")}

    nc = _get_nc()
    in_maps = []
    for c in range(NCORES):
        m = dict(common)
        m["x"] = np.ascontiguousarray(x[c * BL:(c + 1) * BL])
        in_maps.append(m)

    res = run_bass_kernel_spmd(nc, in_maps, list(range(NCORES)))
    out = np.concatenate([res.results[c]["y"] for c in range(NCORES)], axis=0)
    return out.astype(np.float32)


# revision 3
# speedup vs baseline: 1.0557x; 1.0557x over previous
"""Trainium2 Bass kernel for nn_DSFE (dual attention + LFE depthwise-conv block).

Sharding: pure data-parallel over batch B=16 across 8 NeuronCores (2 batches
per core), no collectives. Each core runs the whole per-batch network on its
shard.

v2: mixed-precision rewrite of the fp32 baseline.
  - attention branch end-to-end bf16 matmul inputs (PE 1 cyc/row incl. the
    small-free-dim matmuls and all transposes via a bf16 identity; fp32 was
    4 cyc/row small / 2 cyc/row transpose)
  - LFE branch in fp8e4m3 with DoubleRow matmuls (2 contraction rows per
    partition, 0.5 cyc/row): fc1 and fc2 pair adjacent K-chunks; the 3x3
    depthwise conv pairs taps (0,kw)+(2,kw) whose flat shifts differ by 64
    elements (DoubleRow needs 16B-aligned pair strides; row-1 taps stay
    single fp8 matmuls)
  - weights quantized x64 into fp8 (0.02-scale values are subnormal in
    e4m3 otherwise); T stored as 8*T, GELU output as 64*g via the
    x*sigmoid(1.702x) identity so the x64 rides the DVE multiply for free;
    all scales unwound in PSUM-eviction scale factors (512 for conv, 4096
    for fc2)
  - all weights transposed on-device once in the preamble and kept
    resident in SBUF (bf16/fp8 halve the footprint; no DRAM scratch
    round-trip, no per-batch weight reloads)
  - v_sa never materialized: v_proj = Wv_sa @ (F @ X)^T via a rank-16
    side matmul, so QKVV computes only 3C columns
  - biases via DMA-broadcast const tiles + DVE adds at PSUM eviction
    (replaces rank-1 ones matmuls on the PE)
  - conv wrap-column fixups and the z-eviction moved to GPSIMD

Measured numpy-emulated accuracy of this scheme: rel ~1.1e-2 (gate 2e-2).
"""

from contextlib import ExitStack

import numpy as np

import concourse.bass as bass
import concourse.mybir as mybir
import concourse.tile as tile
from concourse.masks import make_identity

FP = mybir.dt.float32
FR = mybir.dt.float32r
BF = mybir.dt.bfloat16
F8 = mybir.dt.float8e4
AF = mybir.ActivationFunctionType
ALU = mybir.AluOpType
DR = mybir.MatmulPerfMode.DoubleRow
ts = bass.ts

B, W_, H_, C = 16, 32, 32, 512
HEADS, PP, HID = 4, 16, 2048
N = H_ * W_            # 1024
D = C // HEADS         # 128
NCORES = 8
BL = B // NCORES       # 2 batches per core
NCH = N // 128         # 8
CCH = C // 128         # 4
OCH = HID // 128       # 16

NORM_EPS = 1e-12
WSC = 64.0             # fp8 weight scale
TSC = 8.0              # fp8 T scale
GSC = 64.0             # fp8 g scale

# conv input layout: rows padded to stride RS=40 (pair deltas stay 16B
# aligned and no shift ever wraps into a neighbouring row), region A = 8*T,
# region B = A shifted by +2 (pairs row-1 taps), region C = constant 8.0
# (pairs the dw-bias "tap" with (1,1)). Tap pair order (blocks 2p, 2p+1 of
# the diag tile): (0,kw)+(2,kw) for kw=0..2, (1,0)+(1,2), (1,1)+bias.
RS = 40
LEAD = 48
AW = 1392                  # region width (A at 0, B at AW, C at 2*AW)
TAP_ORDER = [(0, 0), (2, 0), (0, 1), (2, 1), (0, 2), (2, 2),
             (1, 0), (1, 2), (1, 1)]


def ap3(t2d, off, pair_step, inner):
    """From a 2D tile AP, build a (P, 2, inner) AP with a custom pair
    stride (for DoubleRow over two shifted windows of the same tile)."""
    a = t2d.copy()
    dims = list(a.ap)
    return bass.AP(a.tensor, a.offset + off,
                   [list(dims[0]), [pair_step, 2], [1, inner]])


def ap4(t2d, off, pair_step, nrow):
    """(P, 2, nrow, 32) AP over the padded conv layout: pair stride +
    row-stride RS + contiguous 32 columns."""
    a = t2d.copy()
    dims = list(a.ap)
    return bass.AP(a.tensor, a.offset + off,
                   [list(dims[0]), [pair_step, 2], [RS, nrow], [1, W_]])


def split_multi_waits(nc):
    """This environment's walrus build encodes at most ONE sync wait per
    instruction; re-host excess waits onto same-engine NoOps just before."""
    n_split = 0
    for f in nc.m.functions:
        for blk in f.blocks:
            new = []
            changed = False
            for inst in blk.instructions:
                si = inst.sync_info
                waits = list(si.on_wait) if si and si.on_wait else []
                if len(waits) > 1:
                    n_split += 1
                    changed = True
                    for w in waits[:-1]:
                        new.append(mybir.InstNoOp(
                            name=nc.get_next_instruction_name(),
                            sync_info=mybir.SyncInfo(on_wait=[w], on_update=[]),
                            bass_nofuse=True,
                            engine=inst.engine,
                        ))
                    inst.sync_info = mybir.SyncInfo(
                        on_wait=[waits[-1]],
                        on_update=list(si.on_update) if si.on_update else [],
                    )
                new.append(inst)
            if changed:
                blk.instructions = new
    return n_split


def build(split_waits=True, loop=1, phases="AB"):
    """Build the per-core Bass module (SPMD: identical program on 8 cores)."""
    nc = bass.Bass("TRN2", target_bir_lowering=False, debug=False)

    def din(name, shape):
        return nc.dram_tensor(name, list(shape), FP, kind="ExternalInput").ap()

    # tiny DRAM scratch to round-trip computed bias rows for partition
    # broadcast (DMA broadcast must source from DRAM)
    scratch = dict(
        brf=nc.dram_tensor("brf_s", [1, C], FP).ap(),
    )

    aps = dict(
        x=din("x", (BL, W_, H_, C)),
        qkvv_w=din("qkvv_w", (4 * C, C)),
        e_w=din("E_w", (PP, N)),
        e_b=din("E_b", (PP,)),
        f_w=din("F_w", (PP, N)),
        f_b=din("F_b", (PP,)),
        temp=din("temp", (HEADS, 1, 1)),
        temp2=din("temp2", (HEADS, 1, 1)),
        out_w=din("out_w", (C // 2, C)),
        out_b=din("out_b", (C // 2,)),
        out2_w=din("out2_w", (C // 2, C)),
        out2_b=din("out2_b", (C // 2,)),
        fc1_w=din("fc1_w", (HID, C)),
        fc1_b=din("fc1_b", (HID,)),
        dw_w=din("dw_w", (HID, 1, 3, 3)),
        dw_b=din("dw_b", (HID,)),
        fc2_w=din("fc2_w", (C, HID)),
        fc2_b=din("fc2_b", (C,)),
        y=nc.dram_tensor("y", [BL, W_, H_, C], FP, kind="ExternalOutput").ap(),
    )

    with tile.TileContext(nc) as tc:
        _emit(nc, tc, aps, scratch, loop, phases)

    if split_waits:
        split_multi_waits(nc)
    return nc


def _transpose_weight(nc, tc, ident, nat_ap, dst_fn, label, scale=None):
    """PE-transpose a natural (R, K) weight; for each K-chunk ci the
    transposed (128, R) block is evicted through dst_fn(ci) -> AP.
    scale!=None evicts via ACT with that scale (for fp8 targets)."""
    dve = nc.vector
    pe = nc.tensor
    R, K = nat_ap.shape
    RC, KC = R // 128, K // 128
    with ExitStack() as es:
        nat_pool = es.enter_context(tc.tile_pool(name=f"tw_nat_{label}", bufs=2))
        ps_pool = es.enter_context(
            tc.tile_pool(name=f"tw_ps_{label}", bufs=4, space="PSUM"))
        for rj in range(RC):
            nat = nat_pool.tile([128, K], FP, tag="nat", name=f"{label}nat{rj}")
            nc.sync.dma_start(nat[:], nat_ap[rj * 128:(rj + 1) * 128])
            for ci in range(KC):
                ps = ps_pool.tile([128, 128], FP, tag="tp", name=f"{label}tp")
                pe.transpose(ps[:], nat[:, ts(ci, 128)], ident[:])
                dst = dst_fn(ci)[:, rj * 128:(rj + 1) * 128]
                if scale is None:
                    dve.tensor_copy(dst, ps[:])
                else:
                    nc.scalar.activation(dst, ps[:], AF.Identity, scale=scale)


def _emit(nc, tc, aps, scratch, loop=1, phases="AB"):
    dve = nc.vector
    pe = nc.tensor

    with ExitStack() as es:
        const = es.enter_context(tc.tile_pool(name="const", bufs=1))

        ident = const.tile([128, 128], FP)
        make_identity(nc, ident)
        identb = const.tile([128, 128], BF)
        make_identity(nc, identb)
        ones_bf = const.tile([1, 128], BF)
        dve.memset(ones_bf[:], 1.0)

        # ---- resident transposed weights
        # qw[i]: (c-chunk i, 4C) bf16  (v_sa quarter used only by v_proj)
        qw = [const.tile([128, 4 * C], BF, tag=f"qw{i}", name=f"qw{i}")
              for i in range(CCH)]
        _transpose_weight(nc, tc, ident, aps["qkvv_w"],
                          lambda ci: qw[ci][:], "qkvv")
        # ft8[j]: fc1T fp8 x64 pair tiles (128, 2*HID): slot s = c-chunk 2j+s
        ft8 = [const.tile([128, 2 * HID], F8, tag=f"ft8{j}", name=f"ft8{j}")
               for j in range(CCH // 2)]
        _transpose_weight(
            nc, tc, ident, aps["fc1_w"],
            lambda ci: ft8[ci // 2][:, (ci % 2) * HID:(ci % 2 + 1) * HID],
            "fc1", scale=WSC)
        # f2t8[j]: fc2T fp8 x64 pair tiles (128, 2*C): slot s = o-chunk 2j+s
        f2t8 = [const.tile([128, 2 * C], F8, tag=f"f2t8{j}", name=f"f2t8{j}")
                for j in range(OCH // 2)]
        _transpose_weight(
            nc, tc, ident, aps["fc2_w"],
            lambda ci: f2t8[ci // 2][:, (ci % 2) * C:(ci % 2 + 1) * C],
            "fc2", scale=WSC)
        # out_w^T / out2_w^T chunks (c-part, j) bf16
        owt = [const.tile([128, C // 2], BF, tag=f"owt{q}", name=f"owt{q}")
               for q in range(CCH)]
        owt2 = [const.tile([128, C // 2], BF, tag=f"owt2{q}", name=f"owt2{q}")
                for q in range(CCH)]
        _transpose_weight(nc, tc, ident, aps["out_w"],
                          lambda ci: owt[ci][:], "outw")
        _transpose_weight(nc, tc, ident, aps["out2_w"],
                          lambda ci: owt2[ci][:], "outw2")

        # E^T as (n-part r, n-chunk k, p) bf16; F^T in m-order (m = 32w+h,
        # matching the natural x row order) since it only contracts raw X
        ewt = const.tile([128, NCH, PP], BF)
        fwt = const.tile([128, NCH, PP], BF)
        with ExitStack() as ef:
            ef_pool = ef.enter_context(tc.tile_pool(name="ef_nat", bufs=2))
            ef_ps = ef.enter_context(tc.tile_pool(name="ef_ps", bufs=4, space="PSUM"))
            e_nat = ef_pool.tile([PP, N], FP, tag="nat", name="e_nat")
            nc.sync.dma_start(e_nat[:], aps["e_w"][:])
            f_nat = ef_pool.tile([PP, N], FP, tag="nat", name="f_nat")
            fv3 = f_nat[:].rearrange("p (w h) -> p w h", h=H_)
            sv3 = aps["f_w"].rearrange("p (h w) -> p h w", w=W_)
            for w in range(W_):
                nc.sync.dma_start(fv3[:, w, :], sv3[:, :, w])
            for k in range(NCH):
                ps = ef_ps.tile([128, PP], FP, tag="tp", name="e_tp")
                pe.transpose(ps[:], e_nat[:, ts(k, 128)], ident[0:PP, 0:PP])
                dve.tensor_copy(ewt[:, k, :], ps[:])
                ps2 = ef_ps.tile([128, PP], FP, tag="tp", name="f_tp")
                pe.transpose(ps2[:], f_nat[:, ts(k, 128)], ident[0:PP, 0:PP])
                dve.tensor_copy(fwt[:, k, :], ps2[:])

        # temp/temp2 broadcast to all 128 partitions: (128, HEADS)
        tcol = const.tile([128, HEADS], FP)
        nc.sync.dma_start(
            tcol[:], aps["temp"].rearrange("h a b -> (a b) h").to_broadcast((128, HEADS)))
        t2col = const.tile([128, HEADS], FP)
        nc.sync.dma_start(
            t2col[:], aps["temp2"].rearrange("h a b -> (a b) h").to_broadcast((128, HEADS)))

        # E/F bias rows broadcast over partitions
        ebc = const.tile([128, PP], FP)
        nc.sync.dma_start(ebc[:], aps["e_b"].unsqueeze(0).to_broadcast((128, PP)))
        fbc = const.tile([128, PP], FP)
        nc.sync.dma_start(fbc[:], aps["f_b"].unsqueeze(0).to_broadcast((128, PP)))

        # OUTA free-dim bias row concat(out_b,out2_b)+fc2_b, broadcast via
        # a DRAM round-trip (SBUF-sourced partition broadcast is illegal)
        br_cat = const.tile([1, C], FP)
        nc.sync.dma_start(br_cat[:, 0:C // 2], aps["out_b"].unsqueeze(0))
        nc.sync.dma_start(br_cat[:, C // 2:C], aps["out2_b"].unsqueeze(0))
        fcb_row = const.tile([1, C], FP)
        nc.sync.dma_start(fcb_row[:], aps["fc2_b"].unsqueeze(0))
        brf_row = const.tile([1, C], FP)
        dve.tensor_add(brf_row[:], br_cat[:], fcb_row[:])
        nc.sync.dma_start(scratch["brf"], brf_row[:])
        brf128 = const.tile([128, C], FP)
        nc.sync.dma_start(brf128[:], scratch["brf"].to_broadcast((128, C)))

        # per-partition (o) bias/weight columns for the LFE branch
        fb1 = const.tile([128, OCH], FP)
        nc.sync.dma_start(fb1[:], aps["fc1_b"].rearrange("(k r) -> r k", r=128))
        fb1_8 = const.tile([128, OCH], FP)
        dve.tensor_scalar_mul(fb1_8[:], fb1[:], TSC)
        dwb = const.tile([128, OCH], FP)
        nc.sync.dma_start(dwb[:], aps["dw_b"].rearrange("(k r) -> r k", r=128))
        dww = const.tile([128, OCH * 9], FP)
        for oc in range(OCH):
            nc.sync.dma_start(
                dww[:, oc * 9:(oc + 1) * 9],
                aps["dw_w"][oc * 128:(oc + 1) * 128].rearrange("r a kh kw -> r (a kh kw)"))
        # x64 weights reordered to TAP_ORDER + the x64 dw-bias as block 9
        dwwr = const.tile([128, OCH * 10], FP)
        dv3 = dww[:].rearrange("p (k t) -> p k t", t=9)
        dr3 = dwwr[:].rearrange("p (k t) -> p k t", t=10)
        for t_new, (kh, kw) in enumerate(TAP_ORDER):
            dve.scalar_tensor_tensor(
                dr3[:, :, t_new], dv3[:, :, 3 * kh + kw], WSC,
                dv3[:, :, 3 * kh + kw], op0=ALU.mult, op1=ALU.bypass)
        dve.scalar_tensor_tensor(
            dr3[:, :, 9], dwb[:], WSC, dwb[:], op0=ALU.mult, op1=ALU.bypass)
        # resident fp8 diag(w) tiles, 10 blocks each (9 taps + bias)
        dgs = [const.tile([128, 10 * 128], F8, tag=f"dg{oc}", name=f"dg{oc}")
               for oc in range(OCH)]
        for oc in range(OCH):
            dve.tensor_mul(
                dgs[oc][:].rearrange("p (t j) -> p t j", j=128),
                identb[:].unsqueeze(1).to_broadcast((128, 10, 128)),
                dwwr[:, oc * 10:(oc + 1) * 10].unsqueeze(2)
                    .to_broadcast((128, 10, 128)))

        cst = dict(ident=ident, identb=identb, ones_bf=ones_bf,
                   tcol=tcol, t2col=t2col,
                   ewt=ewt, fwt=fwt, ebc=ebc, fbc=fbc, brf128=brf128,
                   fb1_8=fb1_8, dgs=dgs,
                   qw=qw, ft8=ft8, f2t8=f2t8, owt=owt, owt2=owt2)

        for rep in range(loop):
            for b in range(BL):
                _emit_batch(nc, tc, b, aps, cst,
                            label=f"{rep}_{b}", phases=phases)


def _emit_batch(nc, tc, b, aps, cst, label=None, phases="AB"):
    if label is None:
        label = str(b)
    act = nc.scalar
    dve = nc.vector
    gps = nc.gpsimd
    pe = nc.tensor
    identb = cst["identb"]
    qw = cst["qw"]

    with ExitStack() as bs:
        xc_pool = bs.enter_context(tc.tile_pool(name=f"xc{label}", bufs=1))
        outa_pool = bs.enter_context(tc.tile_pool(name=f"outa{label}", bufs=1))

        # ---- XC (c, n), n = 32h + w: load x[b] rows naturally (m = 32w + h
        # order), bf16-ize, PE-transpose 128x128 blocks, and fix the h/w swap
        # with a permuted SBUF write AP on the evicts (bf16 + fp8 copies).
        # The same pass accumulates XF = F @ X (16, C) for the v_proj path.
        xflat = aps["x"][b].rearrange("w h c -> (w h) c")  # (N, C), m-order
        xc = [xc_pool.tile([128, N], BF, tag=f"xc{i}", name=f"xc{i}")
              for i in range(CCH)]
        x8p = [xc_pool.tile([128, 2 * N], F8, tag=f"x8p{j}", name=f"x8p{j}")
               for j in range(CCH // 2)]
        xft = xc_pool.tile([128, CCH * PP], BF, tag="xft", name="xft")
        with ExitStack() as xl:
            xn_pool = xl.enter_context(tc.tile_pool(name=f"xn{label}", bufs=3))
            xn_ps = xl.enter_context(
                tc.tile_pool(name=f"xnps{label}", bufs=4, space="PSUM"))
            xf_psp = xl.enter_context(
                tc.tile_pool(name=f"xfps{label}", bufs=1, space="PSUM"))
            xf_ps = xf_psp.tile([PP, C], FP, tag="xf", name="xf_ps")
            for kk in range(NCH):
                xn = xn_pool.tile([128, C], FP, tag="xn", name="xn")
                nc.sync.dma_start(xn[:], xflat[kk * 128:(kk + 1) * 128])
                xnb = xn_pool.tile([128, C], BF, tag="xnb", name="xnb")
                act.activation(xnb[:], xn[:], AF.Identity)
                pe.matmul(xf_ps[:], cst["fwt"][:, kk, :], xnb[:],
                          start=(kk == 0), stop=(kk == NCH - 1),
                          skip_group_check=True)
                for i in range(CCH):
                    ps = xn_ps.tile([128, 128], BF, tag="xt", name="xt")
                    pe.transpose(ps[:], xnb[:, ts(i, 128)], identb[:])
                    xcv = xc[i][:].rearrange("p (h w) -> p h w", w=W_)
                    dve.tensor_copy(
                        xcv[:, :, 4 * kk:4 * kk + 4].transpose([0, 2, 1]),
                        ps[:].rearrange("p (a b) -> p a b", b=32))
                    x8v = x8p[i // 2][:, (i % 2) * N:(i % 2 + 1) * N] \
                        .rearrange("p (h w) -> p h w", w=W_)
                    act.activation(
                        x8v[:, :, 4 * kk:4 * kk + 4].transpose([0, 2, 1]),
                        ps[:].rearrange("p (a b) -> p a b", b=32), AF.Identity)
            xf_sb = xc_pool.tile([PP, C], BF, tag="xf_sb", name="xf_sb")
            dve.tensor_copy(xf_sb[:], xf_ps[:])
            for i in range(CCH):
                psx = xn_ps.tile([128, PP], BF, tag="xft", name="xft_ps", bufs=2)
                pe.transpose(psx[:], xf_sb[:, ts(i, 128)], identb[0:PP, 0:PP])
                dve.tensor_copy(xft[:, ts(i, PP)], psx[:])

        outa = [outa_pool.tile([128, C], FP, tag=f"outa{m}", name=f"outa{m}")
                for m in range(NCH)]
        if phases != "AB":
            for m in range(NCH):
                nc.vector.memset(outa[m][:], 0.0)
        if "B" not in phases:
            ydst0 = aps["y"][b].rearrange("w h c -> h w c")
            nc.sync.dma_start(ydst0[0], outa[0][0:32, :])

        # ======================= phase A: attention =======================
        if "A" in phases:
          with ExitStack() as pa:
            qkvv_pool = pa.enter_context(tc.tile_pool(name=f"qkvv{label}", bufs=1))
            # PSUM budget (8 banks): big bf16 (128,N) x2 = 2, f512 fp32
            # (128,512) x4 = 4, at bf16 (16,N) x1 = 1, med small x4 ~ 1
            ps_big = pa.enter_context(tc.tile_pool(name=f"psb{label}", bufs=3, space="PSUM"))
            ps_f512 = pa.enter_context(tc.tile_pool(name=f"psf{label}", bufs=2, space="PSUM"))
            ps_at = pa.enter_context(tc.tile_pool(name=f"psat{label}", bufs=1, space="PSUM"))
            ps_med = pa.enter_context(tc.tile_pool(name=f"psm{label}", bufs=2, space="PSUM"))

            # ---- QKVV (n, 2C) bf16: only q|k columns; v_ca^T is computed
            # directly in transposed layout per head, v_sa via the XF path
            qkvv = [qkvv_pool.tile([128, 2 * C], BF, tag=f"qkvv{k}", name=f"qkvv{k}")
                    for k in range(NCH)]
            for k in range(NCH):
                for j in range(2):
                    ps = ps_f512.tile([128, 512], FP, tag="f512")
                    for i in range(CCH):
                        pe.matmul(ps[:], xc[i][:, ts(k, 128)],
                                  qw[i][:, ts(j, 512)],
                                  start=(i == 0), stop=(i == CCH - 1))
                    dve.tensor_copy(qkvv[k][:, ts(j, 512)], ps[:])

            with ExitStack() as ph:
                at_pool = pa.enter_context(tc.tile_pool(name=f"at{label}", bufs=2))
                xca_pool = pa.enter_context(tc.tile_pool(name=f"xca{label}", bufs=1))
                xsa_pool = pa.enter_context(tc.tile_pool(name=f"xsa{label}", bufs=1))

                xsa = [xsa_pool.tile([128, N], BF, tag=f"xsa{q}", name=f"xsa{q}")
                       for q in range(CCH)]
                xca = []

                def headA(h):
                    """Transposes + norm chains for head h (PE: 16
                    transposes; ACT/DVE: the sqrt/recip chains)."""
                    qc, kc = h * 128, C + h * 128
                    s = {}
                    qT_ps = ps_big.tile([128, N], BF, tag="big")
                    for k in range(NCH):
                        pe.transpose(qT_ps[:, ts(k, 128)],
                                     qkvv[k][:, qc:qc + 128], identb[:])
                    kT_ps = ps_big.tile([128, N], BF, tag="big")
                    for k in range(NCH):
                        pe.transpose(kT_ps[:, ts(k, 128)],
                                     qkvv[k][:, kc:kc + 128], identb[:])
                    sq = at_pool.tile([128, N], BF, tag="sq")
                    ssq = at_pool.tile([128, 1], FP, tag="ssq")
                    act.activation(sq[:], qT_ps[:], AF.Square, accum_out=ssq[:])
                    nrmq = at_pool.tile([128, 1], FP, tag="nrmq")
                    act.activation(nrmq[:], ssq[:], AF.Sqrt)
                    invq = at_pool.tile([128, 1], FP, tag="invq")
                    dve.reciprocal(invq[:], nrmq[:])
                    invq_t = at_pool.tile([128, 1], FP, tag="invq_t")
                    dve.tensor_mul(invq_t[:], invq[:], cst["tcol"][:, h:h + 1])
                    qn_t = at_pool.tile([128, N], BF, tag="qn_t")
                    act.activation(qn_t[:], qT_ps[:], AF.Identity, scale=invq[:])
                    sqk = at_pool.tile([128, N], BF, tag="sq")
                    ssqk = at_pool.tile([128, 1], FP, tag="ssqk")
                    act.activation(sqk[:], kT_ps[:], AF.Square, accum_out=ssqk[:])
                    nrmk = at_pool.tile([128, 1], FP, tag="nrmk")
                    act.activation(nrmk[:], ssqk[:], AF.Sqrt)
                    invk = at_pool.tile([128, 1], FP, tag="invk")
                    dve.reciprocal(invk[:], nrmk[:])
                    invk_b = at_pool.tile([128, 1], BF, tag="invk_b")
                    dve.tensor_copy(invk_b[:], invk[:])
                    s.update(invq_t=invq_t, qn_t=qn_t, invk_b=invk_b)
                    return s

                def colsc_pe(h, s):
                    """Column-scale tile via rank-1 matmul; emitted late so
                    the PE never stalls on the invk chain."""
                    ikr_ps = ps_med.tile([1, 128], BF, tag="med")
                    pe.transpose(ikr_ps[:], s["invk_b"][:], identb[:])
                    ikr = at_pool.tile([1, 128], BF, tag="ikr")
                    dve.tensor_copy(ikr[:], ikr_ps[:])
                    colsc_ps = ps_med.tile([128, 128], FP, tag="med")
                    pe.matmul(colsc_ps[:], cst["ones_bf"][:], ikr[:],
                              start=True, stop=True)
                    colsc = at_pool.tile([128, 128], FP, tag="colsc")
                    dve.tensor_copy(colsc[:], colsc_ps[:])
                    s["colsc"] = colsc

                def headB(h, s):
                    qc = h * 128            # q columns in QKVV
                    kc = C + h * 128        # k columns
                    vc = 2 * C + h * 128    # v_ca columns in qkvv_w^T

                    # ---- CA scores S0 = q @ k^T
                    s_ps = ps_med.tile([128, 128], FP, tag="med")
                    for k in range(NCH):
                        pe.matmul(s_ps[:], qkvv[k][:, qc:qc + 128],
                                  qkvv[k][:, kc:kc + 128],
                                  start=(k == 0), stop=(k == NCH - 1))
                    s_sb = at_pool.tile([128, 128], FP, tag="s_sb")
                    dve.scalar_tensor_tensor(s_sb[:], s_ps[:], s["invq_t"][:],
                                             s["colsc"][:], op0=ALU.mult,
                                             op1=ALU.mult)

                    # ---- SA: k_proj (d, p) from qkvv
                    kp_ps = ps_med.tile([128, PP], FP, tag="med")
                    for k in range(NCH):
                        pe.matmul(kp_ps[:], qkvv[k][:, kc:kc + 128],
                                  cst["ewt"][:, k, :],
                                  start=(k == 0), stop=(k == NCH - 1))
                    kp_sb = at_pool.tile([128, PP], BF, tag="kp_sb")
                    dve.tensor_add(kp_sb[:], kp_ps[:], cst["ebc"][:])

                    # ---- v_ca^T (e, n) computed directly
                    vt_sb = at_pool.tile([128, N], BF, tag="vt_sb")
                    for u in range(2):
                        vt_ps = ps_f512.tile([128, 512], FP, tag="f512")
                        for i in range(CCH):
                            pe.matmul(vt_ps[:], qw[i][:, vc:vc + 128],
                                      xc[i][:, ts(u, 512)],
                                      start=(i == 0), stop=(i == CCH - 1))
                        dve.tensor_copy(vt_sb[:, ts(u, 512)], vt_ps[:])

                    # ---- CA row softmax (1/sum folded into x_ca evict)
                    negmax = at_pool.tile([128, 1], FP, tag="negmax")
                    dve.tensor_reduce(negmax[:], s_sb[:], axis=mybir.AxisListType.X,
                                      op=ALU.max, negate=True)
                    e_sb = at_pool.tile([128, 128], BF, tag="e_sb")
                    sume = at_pool.tile([128, 1], FP, tag="sume")
                    act.activation(e_sb[:], s_sb[:], AF.Exp, bias=negmax[:],
                                   accum_out=sume[:])
                    rex = at_pool.tile([128, 1], FP, tag="rex")
                    dve.reciprocal(rex[:], sume[:])

                    # ---- v_proj from Wv_sa @ XF^T
                    vp_ps = ps_med.tile([128, PP], FP, tag="med")
                    for i in range(CCH):
                        pe.matmul(vp_ps[:],
                                  qw[i][:, 3 * C + h * 128:3 * C + (h + 1) * 128],
                                  xft[:, ts(i, PP)],
                                  start=(i == 0), stop=(i == CCH - 1))
                    vp_sb = at_pool.tile([128, PP], BF, tag="vp_sb")
                    dve.tensor_add(vp_sb[:], vp_ps[:], cst["fbc"][:])
                    vpt_ps = ps_med.tile([16, 128], BF, tag="med")
                    pe.transpose(vpt_ps[:], vp_sb[:], identb[:])
                    vpt_sb = at_pool.tile([16, 128], BF, tag="vpt_sb")
                    dve.tensor_copy(vpt_sb[:], vpt_ps[:])

                    et_ps = ps_med.tile([128, 128], BF, tag="med")
                    pe.transpose(et_ps[:], e_sb[:], identb[:])
                    et_sb = at_pool.tile([128, 128], BF, tag="et_sb")
                    dve.tensor_copy(et_sb[:], et_ps[:])

                    # ---- x_ca (d, n) = (1/sum) * exp(S)^T.T @ v_ca^T
                    xca_h = xca_pool.tile([128, N], BF, tag=f"xca{h}")
                    for u in range(2):
                        xca_ps = ps_f512.tile([128, 512], FP, tag="f512")
                        pe.matmul(xca_ps[:], et_sb[:],
                                  vt_sb[:, ts(u, 512)], start=True, stop=True)
                        act.activation(xca_h[:, ts(u, 512)], xca_ps[:],
                                       AF.Identity, scale=rex[:])
                    xca.append(xca_h)

                    # ---- A0 (n, p) per n-chunk, all 8 in one (128, 8, 16)
                    a_ps = ps_med.tile([128, 128], FP, tag="med")
                    a3 = a_ps[:].rearrange("p (k s) -> p k s", s=PP)
                    for k in range(NCH):
                        pe.matmul(a3[:, k, :], s["qn_t"][:, ts(k, 128)],
                                  kp_sb[:], start=True, stop=True)

                    # ---- segmented softmax over p (free-dim broadcasts)
                    amax = at_pool.tile([128, NCH], FP, tag="amax")
                    dve.tensor_reduce(amax[:], a3, axis=mybir.AxisListType.X,
                                      op=ALU.max)
                    zt = at_pool.tile([128, 128], BF, tag="zt")
                    zt3 = zt[:].rearrange("p (k s) -> p k s", s=PP)
                    dve.tensor_sub(zt3, a3,
                                   amax[:].unsqueeze(2).to_broadcast((128, NCH, PP)))
                    ez = at_pool.tile([128, 128], BF, tag="ez")
                    act.activation(ez[:], zt[:], AF.Exp, scale=cst["t2col"][:, h:h + 1])
                    ez3 = ez[:].rearrange("p (k s) -> p k s", s=PP)
                    esum = at_pool.tile([128, NCH], FP, tag="esum")
                    dve.tensor_reduce(esum[:], ez3, axis=mybir.AxisListType.X,
                                      op=ALU.add)
                    rsum = at_pool.tile([128, NCH], FP, tag="rsum")
                    dve.reciprocal(rsum[:], esum[:])
                    attn = at_pool.tile([128, 128], BF, tag="attn")
                    attn3 = attn[:].rearrange("p (k s) -> p k s", s=PP)
                    dve.tensor_mul(attn3, ez3,
                                   rsum[:].unsqueeze(2).to_broadcast((128, NCH, PP)))

                    # ---- attn^T (p, n)
                    at_ps = ps_at.tile([16, N], BF, tag="at")
                    for k in range(NCH):
                        pe.transpose(at_ps[:, ts(k, 128)], attn3[:, k, :], identb[:])
                    at_sb = at_pool.tile([16, N], BF, tag="at_sb")
                    dve.tensor_copy(at_sb[:], at_ps[:])

                    # ---- x_sa in scrambled (c'=n%512, n'=8d+2h+s) layout,
                    # 4 chunks per f512 PSUM tile
                    for sdx in range(2):
                        xs_ps = ps_f512.tile([128, 512], FP, tag="f512")
                        for q in range(CCH):
                            k = 4 * sdx + q
                            pe.matmul(xs_ps[:, ts(q, 128)], at_sb[:, ts(k, 128)],
                                      vpt_sb[:], start=True, stop=True,
                                      skip_group_check=True)
                        for q in range(CCH):
                            dst = xsa[q][:].rearrange("p (d e) -> p d e", e=8)[:, :, 2 * h + sdx]
                            dve.tensor_copy(dst, xs_ps[:, ts(q, 128)])

                # two-head software pipeline: head h+1's transposes/norm
                # chains overlap head h's main block on the other engines
                sts = {0: headA(0)}
                for h in range(HEADS):
                    if h + 1 < HEADS:
                        sts[h + 1] = headA(h + 1)
                    colsc_pe(h, sts[h])
                    headB(h, sts.pop(h))

                # ---- OUTA (n, 512) = [x_sa@out_w^T | x_ca@out2_w^T] (+bias
                # at the DVE evict)
                for m in range(NCH):
                    o_ps = ps_f512.tile([128, C], FP, tag="f512")
                    for q in range(CCH):
                        pe.matmul(o_ps[:, 0:C // 2], xsa[q][:, ts(m, 128)],
                                  cst["owt"][q][:], start=(q == 0), stop=(q == CCH - 1),
                                  skip_group_check=True)
                    for h in range(HEADS):
                        pe.matmul(o_ps[:, C // 2:C], xca[h][:, ts(m, 128)],
                                  cst["owt2"][h][:], start=(h == 0), stop=(h == HEADS - 1),
                                  skip_group_check=True)
                    dve.tensor_add(outa[m][:], o_ps[:, 0:C], cst["brf128"][:])

        # ======================= phase B: LFE =======================
        if "B" in phases:
          with ExitStack() as pb:
            g_pool = pb.enter_context(tc.tile_pool(name=f"g{label}", bufs=1))
            # PSUM: t1/f_ps tag "big" 2 banks x2 bufs + conv (128,1280) 3
            # banks x1 = 7 banks
            ps_big2 = pb.enter_context(tc.tile_pool(name=f"psb2{label}", bufs=2, space="PSUM"))
            ps_conv = pb.enter_context(tc.tile_pool(name=f"psc{label}", bufs=1, space="PSUM"))
            # g8 pair tiles (128, 2, N) fp8: slot = oc % 2
            g8p = [g_pool.tile([128, 2 * N], F8, tag=f"g8p{j}", name=f"g8p{j}")
                   for j in range(OCH // 2)]

            with ExitStack() as pf1:
                conv_pool = pf1.enter_context(tc.tile_pool(name=f"conv{label}", bufs=2))
                # two explicit padded-T buffers alternated across ocs (ONE
                # logical tensor each, so the once-per-batch pad/C memsets
                # stay visible to every oc's reads)
                t_sb2 = [conv_pool.tile([128, 3 * AW], F8, tag=f"tsb{i}",
                                        name=f"tsb{i}") for i in range(2)]
                for i in range(2):
                    gps.memset(t_sb2[i][:, 0:2 * AW], 0.0)
                    gps.memset(t_sb2[i][:, 2 * AW:3 * AW], TSC)
                t_sbs = {}

                def emit_fc1(oc):
                    # fc1 via fp8 DoubleRow (c-chunk pairs); T stored as 8*T
                    # into padded region A (row stride RS)
                    t1_ps = ps_big2.tile([128, N], FP, tag="big")
                    for u in (0, 512):
                        for j in range(CCH // 2):
                            pe.matmul(
                                t1_ps[:, u:u + 512],
                                cst["ft8"][j][:]
                                    .rearrange("p (i f) -> p i f", i=2)[:, :, ts(oc, 128)],
                                ap3(x8p[j][:], u, N, 512),
                                start=(j == 0), stop=(j == CCH // 2 - 1),
                                perf_mode=DR)
                    t_sb = t_sb2[oc % 2]
                    act.activation(
                        t_sb[:, LEAD:LEAD + RS * H_]
                            .rearrange("p (h q) -> p h q", q=RS)[:, :, 0:W_],
                        t1_ps[:].rearrange("p (h w) -> p h w", w=W_),
                        AF.Identity, scale=TSC / WSC,
                        bias=cst["fb1_8"][:, oc:oc + 1])
                    # region B (= A shifted by +2, pairs the row-1 taps)
                    # written straight from the fc1 PSUM on the DVE, in
                    # parallel with ACT's region-A write
                    dve.scalar_tensor_tensor(
                        t_sb[:, AW + LEAD - 2:AW + LEAD - 2 + RS * H_]
                            .rearrange("p (h q) -> p h q", q=RS)[:, :, 0:W_],
                        t1_ps[:].rearrange("p (h w) -> p h w", w=W_),
                        TSC / WSC,
                        cst["fb1_8"][:, oc:oc + 1].unsqueeze(2)
                            .to_broadcast((128, H_, W_)),
                        op0=ALU.mult, op1=ALU.add)
                    t_sbs[oc] = t_sb

                def emit_conv(oc):
                    # 3x3 depthwise conv + dw-bias in DoubleRow matmuls over
                    # the padded layout; the output stays in PADDED (RS-wide
                    # row) coordinates in PSUM so every pair read is one
                    # contiguous 320-col window (8 image rows per matmul).
                    # Pairs: (0,kw)+(2,kw) at delta 2*RS; (1,0)+(1,2) pairs
                    # A with B (delta AW); (1,1)+bias pairs A with the
                    # constant-8.0 region C. PSUM holds 512*(conv + dw_b).
                    t_sb = t_sbs.pop(oc)
                    dg = cst["dgs"][oc]
                    conv_ps = ps_conv.tile([128, RS * H_], FP, tag="conv",
                                           name="conv_ps")
                    deltas = [(LEAD - RS - 1, 2 * RS), (LEAD - RS, 2 * RS),
                              (LEAD - RS + 1, 2 * RS), (LEAD - 1, AW),
                              (LEAD, 2 * AW)]
                    # chunks bank-aligned (matmul out must stay in one bank);
                    # reads stay contiguous for any padded-col range
                    for c0, cw in ((0, 512), (512, 512), (1024, 256)):
                        for p, (off0, dlt) in enumerate(deltas):
                            pe.matmul(
                                conv_ps[:, c0:c0 + cw],
                                dg[:].rearrange("p (i j) -> p i j", i=10)
                                    [:, 2 * p:2 * p + 2, :],
                                ap3(t_sb[:], off0 + c0, dlt, cw),
                                start=(p == 0), stop=(p == len(deltas) - 1),
                                perf_mode=DR, skip_group_check=True)

                    # GELU via x*sigmoid(1.702x); g stored as 64*g fp8, the
                    # z factor read straight from PSUM in the DVE multiply
                    cpv = conv_ps[:].rearrange("p (h q) -> p h q", q=RS)[:, :, 0:W_]
                    sg = conv_pool.tile([128, N], BF, tag="sg", bufs=2)
                    act.activation(sg[:].rearrange("p (h w) -> p h w", w=W_),
                                   cpv, AF.Sigmoid, scale=1.702 / (TSC * WSC))
                    dve.scalar_tensor_tensor(
                        g8p[oc // 2][:, (oc % 2) * N:(oc % 2 + 1) * N]
                            .rearrange("p (h w) -> p h w", w=W_),
                        cpv, GSC / (TSC * WSC),
                        sg[:].rearrange("p (h w) -> p h w", w=W_),
                        op0=ALU.mult, op1=ALU.mult)

                # two-stage software pipeline: fc1(oc+1) issues on the PE
                # while ACT evicts 8*T for conv(oc), hiding the handoff
                for oc in range(OCH):
                    emit_fc1(oc)
                    if oc > 0:
                        emit_conv(oc - 1)
                emit_conv(OCH - 1)

            # ---- fc2 (fp8 DoubleRow over o-chunk pairs) + OUTA -> y
            with ExitStack() as pf2:
                fin_pool = pf2.enter_context(tc.tile_pool(name=f"fin{label}", bufs=2))

                ydst = aps["y"][b].rearrange("w h c -> h w c")  # (H, W, C)
                for m in range(NCH):
                    f_ps2 = ps_big2.tile([128, N], FP, tag="big")
                    f_ps = f_ps2[:, 0:C]
                    for j in range(OCH // 2):
                        pe.matmul(
                            f_ps,
                            g8p[j][:].rearrange("p (i f) -> p i f", i=2)[:, :, ts(m, 128)],
                            cst["f2t8"][j][:].rearrange("p (i f) -> p i f", i=2),
                            start=(j == 0), stop=(j == OCH // 2 - 1),
                            perf_mode=DR)
                    fin = fin_pool.tile([128, C], FP, tag="fin")
                    dve.scalar_tensor_tensor(fin[:], f_ps, 1.0 / (GSC * WSC),
                                             outa[m][:], op0=ALU.mult, op1=ALU.add)
                    for g in range(4):
                        h_row = 4 * m + g
                        nc.sync.dma_start(ydst[h_row], fin[32 * g:32 * (g + 1), :])


_BUILD_CACHE = {}


def _get_nc():
    if "nc" not in _BUILD_CACHE:
        _BUILD_CACHE["nc"] = build()
    return _BUILD_CACHE["nc"]


def kernel(**inputs):
    from concourse.bass_utils import run_bass_kernel_spmd

    def f32(a):
        return np.ascontiguousarray(np.asarray(a, dtype=np.float32))

    x = f32(inputs["x"])
    assert x.shape == (B, W_, H_, C), x.shape
    common = {k: f32(inputs[k]) for k in
              ("qkvv_w", "E_w", "E_b", "F_w", "F_b", "temp", "temp2",
               "out_w", "out_b", "out2_w", "out2_b",
               "fc1_w", "fc1_b", "dw_w", "dw_b", "fc2_w", "fc2_b")}

    nc = _get_nc()
    in_maps = []
    for c in range(NCORES):
        m = dict(common)
        m["x"] = np.ascontiguousarray(x[c * BL:(c + 1) * BL])
        in_maps.append(m)

    res = run_bass_kernel_spmd(nc, in_maps, list(range(NCORES)))
    out = np.concatenate([res.results[c]["y"] for c in range(NCORES)], axis=0)
    return out.astype(np.float32)
